# revision 2
# baseline (speedup 1.0000x reference)
"""Trainium2 Bass kernel for BalSupMoCoNet supervised-contrastive loss.

Triangle-symmetric decomposition (sim matrix is symmetric, so each unordered
tile pair is computed once):
  N = 16640 rows = 130 tiles of 128. Core c owns row-tiles a = 8v + c
  (v = 0..16). Row a computes sim tiles (a, t) for t >= a: row sums via ACT
  accum_out, column sums via DVE accumulation of bf16 exp tiles into colacc,
  partition-reduced per 128-col tile by PE ones-matmuls. Per-core partial
  row/col sums are combined with an on-device AllReduce ([128, 260] f32),
  then each core runs the epilogue for its own rows; host sums 8 scalars.

SPMD uniformity: all cores run an identical program. Per-core variation comes
only through input data: colbuf_c = [featT[:, c*128:] | zeros(c*128)] shifts
the column space so row v's window is always local cols [8v*128, 16640).
The c*128 zero pad columns add a deterministic 128*exp(-C) per row to the
row sums (subtracted via the host-provided m_corr) and land their column
sums in local tiles >= 130-c, which the masked local->global scatter drops.
The diagonal tile is always the first 128 columns of each window; the DVE
colacc add skips it uniformly (its contribution is fully inside the row sum).
"""

import sys

import numpy as np

try:
    import concourse.bass as bass
except ImportError:
    sys.path.insert(0, "/opt/trn_rl_repo")
    import concourse.bass as bass

import concourse.bacc as bacc
import concourse.tile as tile
from concourse import mybir
from concourse.bass_utils import run_bass_kernel_spmd

AF = mybir.ActivationFunctionType
ALU = mybir.AluOpType
F32 = mybir.dt.float32
BF16 = mybir.dt.bfloat16


class Cfg:
    def __init__(self, B=256, K=8192, ncores=8):
        self.B, self.K, self.D, self.ncores = B, K, 128, ncores
        self.T = 0.07
        self.C = 1.0 / self.T
        self.N = B + 2 * K                  # 16640
        self.NT = self.N // 128             # 130 row/col tiles
        self.VR = 17                        # row slots per core (8*17=136>=130)
        self.CHW = 2048                     # psum/ACT chunk width
        self.MMW = 512                      # matmul width (1 psum bank)
        self.MAXCH = 9                      # max chunks per row (v=0)


FULL = Cfg()


def _row_chunks(cfg, v):
    """Chunks (k, start, w) of row-slot v's window [8v*128, N)."""
    start0 = 8 * v * 128
    W = cfg.N - start0
    out = []
    k = 0
    while k * cfg.CHW < W:
        w = min(cfg.CHW, W - k * cfg.CHW)
        out.append((k, start0 + k * cfg.CHW, w))
        k += 1
    return out


def build_program(cfg):
    nc = bacc.Bacc("TRN2", target_bir_lowering=False, debug=False,
                   enable_asserts=True, num_devices=cfg.ncores)

    N, NT, VR, CHW, MMW = cfg.N, cfg.NT, cfg.VR, cfg.CHW, cfg.MMW
    T, C = cfg.T, cfg.C

    d_colbuf = nc.dram_tensor("colbuf", [128, N], BF16, kind="ExternalInput").ap()
    d_g01 = nc.dram_tensor("g01", [128, 2], BF16, kind="ExternalInput").ap()
    d_eii = nc.dram_tensor("m_eii", [128, NT], F32, kind="ExternalInput").ap()
    d_mw = nc.dram_tensor("m_w", [128, NT], F32, kind="ExternalInput").ap()
    d_i1l = nc.dram_tensor("m_i1l", [128, VR], F32, kind="ExternalInput").ap()
    d_rsl = nc.dram_tensor("m_rsl", [128, VR], F32, kind="ExternalInput").ap()
    d_bl = nc.dram_tensor("m_bl", [128, VR], F32, kind="ExternalInput").ap()
    d_wl = nc.dram_tensor("m_wl", [128, VR], F32, kind="ExternalInput").ap()
    d_corr = nc.dram_tensor("m_corr", [128, VR], F32, kind="ExternalInput").ap()
    d_oh = nc.dram_tensor("onehot", [128, cfg.ncores], F32, kind="ExternalInput").ap()
    d_out = nc.dram_tensor("out", [128, 1], F32, kind="ExternalOutput").ap()

    from contextlib import ExitStack
    with tile.TileContext(nc) as tc, ExitStack() as ctx:
        feat = ctx.enter_context(tc.tile_pool(name="feat", bufs=1))
        consts = ctx.enter_context(tc.tile_pool(name="consts", bufs=1))
        accs = ctx.enter_context(tc.tile_pool(name="accs", bufs=1))
        epool = ctx.enter_context(tc.tile_pool(name="epool", bufs=2))
        pspool = ctx.enter_context(tc.tile_pool(name="psum", bufs=2, space="PSUM"))
        dram = ctx.enter_context(tc.tile_pool(name="dram", bufs=2, space="DRAM"))

        # ---- input DMAs (colbuf split so compute starts after chunk 0) ----
        colbuf = feat.tile([128, N], BF16, tag="colbuf")
        for k in range(cfg.MAXCH):
            lo = k * CHW
            hi = min(N, lo + CHW)
            nc.sync.dma_start(out=colbuf[:, lo:hi], in_=d_colbuf[:, lo:hi])
        g01 = consts.tile([128, 2], BF16, tag="g01")
        nc.sync.dma_start(out=g01[:], in_=d_g01[:])
        m_eii = consts.tile([128, NT], F32, tag="meii")
        nc.sync.dma_start(out=m_eii[:], in_=d_eii[:])
        m_w = consts.tile([128, NT], F32, tag="mw")
        nc.sync.dma_start(out=m_w[:], in_=d_mw[:])
        m_i1l = consts.tile([128, VR], F32, tag="mi1l")
        nc.sync.dma_start(out=m_i1l[:], in_=d_i1l[:])
        m_rsl = consts.tile([128, VR], F32, tag="mrsl")
        nc.sync.dma_start(out=m_rsl[:], in_=d_rsl[:])
        m_bl = consts.tile([128, VR], F32, tag="mbl")
        nc.sync.dma_start(out=m_bl[:], in_=d_bl[:])
        m_wl = consts.tile([128, VR], F32, tag="mwl")
        nc.sync.dma_start(out=m_wl[:], in_=d_wl[:])
        m_corr = consts.tile([128, VR], F32, tag="mcorr")
        nc.sync.dma_start(out=m_corr[:], in_=d_corr[:])
        onehot = consts.tile([128, cfg.ncores], F32, tag="oh")
        nc.sync.dma_start(out=onehot[:], in_=d_oh[:])

        b_negC = consts.tile([128, 1], F32, tag="negC")
        nc.vector.memset(b_negC[:], -C)
        ones_bf = consts.tile([128, 1], BF16, tag="ones")
        nc.vector.memset(ones_bf[:], 1.0)

        # ---- accumulators ----
        colacc = accs.tile([128, N], BF16, tag="colacc")
        nc.vector.memset(colacc[:], 0.0)
        rowsums = accs.tile([128, VR * cfg.MAXCH], F32, tag="rowsums")
        nc.vector.memset(rowsums[:], 0.0)
        colp = accs.tile([128, NT], F32, tag="colp")        # local-layout col sums
        nc.vector.memset(colp[:], 0.0)
        # sp = globally-aligned rowparts+colparts (summed), the AR payload
        sp = accs.tile([128, NT], F32, tag="sp")
        nc.vector.memset(sp[:], 0.0)

        # ---- main loop ----
        def colsum_batch(tiles, cps, off):
            """Partition-reduce finalized colacc tiles into colp via PE,
            using spare columns [off, off+len) of the active psum tile."""
            tiles = [t for t in tiles if t > 0]   # local tile 0 is never written
            if not tiles:
                return
            for i, t in enumerate(tiles):
                nc.tensor.matmul(cps[:, off + i:off + i + 1],
                                 colacc[:, t * 128:(t + 1) * 128],
                                 ones_bf[:], start=True, stop=True)
            t0 = tiles[0]
            nc.vector.tensor_copy(colp[:, t0:t0 + len(tiles)],
                                  cps[:, off:off + len(tiles)])

        def colp_scatter(lo, hi):
            """Scatter colp local tiles [lo, hi) into sp (global tile = t+cc)."""
            for cc in range(cfg.ncores):
                glo, ghi = lo + cc, min(hi + cc, NT)
                if ghi <= glo:
                    continue
                nc.vector.scalar_tensor_tensor(
                    sp[:, glo:ghi], colp[:, lo:ghi - cc], onehot[:, cc:cc + 1],
                    sp[:, glo:ghi], op0=ALU.mult, op1=ALU.add)

        for v in range(VR):
            lhsT = colbuf[:, 8 * v * 128:(8 * v + 1) * 128]
            chunks = _row_chunks(cfg, v)
            for k, start, w in chunks:
                ps = pspool.tile([128, CHW], F32, tag="ps")
                for j in range(-(-w // MMW)):
                    mw = min(MMW, w - j * MMW)
                    nc.tensor.matmul(ps[:, j * MMW:j * MMW + mw], lhsT,
                                     colbuf[:, start + j * MMW:start + j * MMW + mw],
                                     start=True, stop=True)
                et = epool.tile([128, CHW], BF16, tag="et")
                nc.scalar.activation(et[:, 0:w], ps[:, 0:w], AF.Exp,
                                     bias=b_negC[:], scale=1.0 / T,
                                     accum_out=rowsums[:, v * cfg.MAXCH + k:
                                                       v * cfg.MAXCH + k + 1])
                skip = 128 if k == 0 else 0
                if w > skip:
                    nc.vector.tensor_add(colacc[:, start + skip:start + w],
                                         colacc[:, start + skip:start + w],
                                         et[:, skip:w])
            # column tiles [8v+1, 8v+8] got their last contribution from row v;
            # reduce them into the spare columns of this row's last psum tile
            lo = 8 * v + 1 if v > 0 else 0
            hi = min(8 * v + 8, NT - 1)
            w_last = chunks[-1][2]
            colsum_batch(list(range(lo, hi + 1)), ps, w_last)
            if v == 8:
                colp_scatter(0, 65)          # wave 1: tiles final by row 8

        # ---- dg dots + AR-independent epilogue pieces (nothing overlaps the
        # collective in this environment, so do all of it before the AR) ----
        dgps = pspool.tile([128, CHW], F32, tag="ps")
        for v in range(VR):
            nc.tensor.matmul(dgps[:, 2 * v:2 * v + 2],
                             colbuf[:, 8 * v * 128:(8 * v + 1) * 128],
                             g01[:], start=True, stop=True)
        dgl = accs.tile([128, 2 * VR], F32, tag="dgl")
        nc.vector.tensor_copy(dgl[:], dgps[:, 0:2 * VR])
        # local S2-part of the loss: el = ((dg1-dg0)*i1 + dg0)*rs + bl, masked
        el = accs.tile([128, VR], F32, tag="el")
        nc.vector.tensor_sub(el[:], dgl[:, 1:2 * VR:2], dgl[:, 0:2 * VR:2])
        nc.vector.tensor_mul(el[:], el[:], m_i1l[:])
        nc.vector.tensor_add(el[:], el[:], dgl[:, 0:2 * VR:2])
        nc.vector.tensor_mul(el[:], el[:], m_rsl[:])
        nc.vector.tensor_add(el[:], el[:], m_bl[:])
        nc.vector.tensor_mul(el[:], el[:], m_wl[:])
        outv1 = accs.tile([128, 1], F32, tag="outv1")
        nc.vector.reduce_sum(outv1[:], el[:], axis=mybir.AxisListType.X)

        # ---- local row sums: reduce chunks, subtract pad-zero correction ----
        rowp = accs.tile([128, VR], F32, tag="rowp")
        nc.vector.reduce_sum(rowp[:], rowsums[:].rearrange("p (v k) -> p v k",
                                                           k=cfg.MAXCH),
                             axis=mybir.AxisListType.X)
        nc.vector.tensor_sub(rowp[:], rowp[:], m_corr[:])

        # scatter row sums into sp, finish col sum scatter
        for cc in range(cfg.ncores):
            nvs = len(range(cc, NT, 8))          # 17 for cc<2 else 16
            nc.vector.scalar_tensor_tensor(
                sp[:, cc:NT:8], rowp[:, 0:nvs], onehot[:, cc:cc + 1],
                sp[:, cc:NT:8], op0=ALU.mult, op1=ALU.add)
        colp_scatter(65, NT)                     # wave 2

        # ---- cross-core AllReduce of the combined partial sums ----
        ar_in = dram.tile([128, NT], F32)
        ar_out = dram.tile([128, NT], F32)
        nc.sync.dma_start(out=ar_in[:], in_=sp[:])
        nc.gpsimd.collective_compute(
            "AllReduce", ALU.add,
            replica_groups=[list(range(cfg.ncores))],
            ins=[ar_in.opt()], outs=[ar_out.opt()])

        # ---- post-AR epilogue: -log(S1) part only ----
        artot = accs.tile([128, NT], F32, tag="artot")
        nc.sync.dma_start(out=artot[:], in_=ar_out[:])
        nc.vector.tensor_sub(artot[:], artot[:], m_eii[:])   # S1
        lg = accs.tile([128, NT], F32, tag="lg")
        nc.scalar.activation(lg[:], artot[:], AF.Ln)
        nc.vector.tensor_mul(lg[:], lg[:], m_w[:])           # -log(S1) * (-1/N)
        outv = accs.tile([128, 1], F32, tag="outv")
        nc.vector.reduce_sum(outv[:], lg[:], axis=mybir.AxisListType.X)
        nc.vector.tensor_sub(outv[:], outv1[:], outv[:])
        nc.sync.dma_start(out=d_out[:], in_=outv[:])

    nc.compile()
    return nc


def prep_in_maps(cfg, q, ba_queue, nonba_queue, targets):
    q = np.ascontiguousarray(np.asarray(q), dtype=np.float32)
    ba = np.asarray(ba_queue, dtype=np.float32)
    nb = np.asarray(nonba_queue, dtype=np.float32)
    tg = np.asarray(targets).astype(np.int64)
    B, K, N, NT, VR = cfg.B, cfg.K, cfg.N, cfg.NT, cfg.VR

    import ml_dtypes
    BF = ml_dtypes.bfloat16

    qn = q / np.clip(np.linalg.norm(q, axis=1, keepdims=True), 1e-12, None)
    featT = np.concatenate([qn.T, ba.T, nb.T], axis=1).astype(BF)   # [128, N]
    feat32 = featT.astype(np.float32)
    labels = np.concatenate([tg, np.ones(K, np.int64), np.zeros(K, np.int64)])
    g0 = feat32.astype(np.float64)[:, labels == 0].sum(axis=1)
    g1 = feat32.astype(np.float64)[:, labels == 1].sum(axis=1)
    g01 = np.stack([g0, g1], axis=1).astype(np.float32).astype(BF)  # [128, 2]
    dvec = (feat32 * feat32).sum(axis=0).astype(np.float32)         # [N]
    c1 = int(labels.sum())
    c0 = N - c1
    P = np.where(labels == 1, c1 - 1, c0 - 1).astype(np.float64)
    rs = (1.0 / (cfg.T * P)).astype(np.float32)

    def tiled(vec):
        return np.ascontiguousarray(
            np.broadcast_to(vec, (N,)).reshape(NT, 128).T.astype(np.float32))

    m_eii = tiled(np.exp(dvec.astype(np.float64) / cfg.T - cfg.C))
    i1f = labels.astype(np.float32)
    bvec = -(dvec * rs + cfg.C)
    expC = float(np.exp(-cfg.C))

    def local_rows(vec, c, fill=0.0):
        """[128, VR] layout for this core's rows a = 8v + c (dummy -> fill)."""
        out = np.full((128, VR), fill, np.float32)
        for v in range(VR):
            t = 8 * v + c
            if t < NT:
                out[:, v] = vec[t * 128:(t + 1) * 128]
        return np.ascontiguousarray(out)

    in_maps = []
    for c in range(cfg.ncores):
        colbuf = np.concatenate(
            [featT[:, c * 128:], np.zeros((128, c * 128), BF)], axis=1)
        wcol = np.where(np.arange(NT) % 8 == c, -1.0 / N, 0.0).astype(np.float32)
        m_w = np.ascontiguousarray(
            np.broadcast_to(wcol, (128, NT)).astype(np.float32))
        wl = np.full((128, VR), -1.0 / N, np.float32)
        if c >= 2:
            wl[:, VR - 1] = 0.0               # dummy slot
        corr = np.full(VR, c * 128 * expC, np.float32)
        if c >= 2:
            corr[VR - 1] = 256.0 * expC       # dummy slot: all-zero window
        m_corr = np.ascontiguousarray(
            np.broadcast_to(corr, (128, VR)).astype(np.float32))
        oh = np.zeros((128, cfg.ncores), np.float32)
        oh[:, c] = 1.0
        in_maps.append({
            "colbuf": np.ascontiguousarray(colbuf),
            "g01": np.ascontiguousarray(g01),
            "m_eii": m_eii,
            "m_w": m_w,
            "m_i1l": local_rows(i1f, c),
            "m_rsl": local_rows(rs, c),
            "m_bl": local_rows(bvec, c),
            "m_wl": np.ascontiguousarray(wl),
            "m_corr": m_corr,
            "onehot": oh,
        })
    return in_maps


_PROGRAM = None


def get_program():
    global _PROGRAM
    if _PROGRAM is None:
        _PROGRAM = build_program(FULL)
    return _PROGRAM


def run_on_hw(in_maps, trace=False):
    nc = get_program()
    return run_bass_kernel_spmd(nc, in_maps, list(range(FULL.ncores)), trace=trace)


def kernel(q, ba_queue, nonba_queue, targets):
    in_maps = prep_in_maps(FULL, q, ba_queue, nonba_queue, targets)
    res = run_on_hw(in_maps)
    total = sum(float(r["out"].astype(np.float64).sum()) for r in res.results)
    return np.array(total, dtype=np.float32)


# revision 3
# speedup vs baseline: 1.0610x; 1.0610x over previous
"""Trainium2 Bass kernel for BalSupMoCoNet supervised-contrastive loss.

Triangle-symmetric decomposition (sim matrix is symmetric, so each unordered
tile pair is computed once):
  N = 16640 rows = 130 tiles of 128. Core c owns row-tiles a = 8v + c
  (v = 0..16). Row a computes sim tiles (a, t) for t >= a: row sums via ACT
  accum_out, column sums via DVE accumulation of bf16 exp tiles into colacc,
  partition-reduced per 128-col tile by PE ones-matmuls. Per-core partial
  row/col sums are combined with an on-device AllReduce ([128, 260] f32),
  then each core runs the epilogue for its own rows; host sums 8 scalars.

SPMD uniformity: all cores run an identical program. Per-core variation comes
only through input data: colbuf_c = [featT[:, c*128:] | zeros(c*128)] shifts
the column space so row v's window is always local cols [8v*128, 16640).
The c*128 zero pad columns add a deterministic 128*exp(-C) per row to the
row sums (subtracted via the host-provided m_corr) and land their column
sums in local tiles >= 130-c, which the masked local->global scatter drops.
The diagonal tile is always the first 128 columns of each window; the DVE
colacc add skips it uniformly (its contribution is fully inside the row sum).
"""

import sys

import numpy as np

try:
    import concourse.bass as bass
except ImportError:
    sys.path.insert(0, "/opt/trn_rl_repo")
    import concourse.bass as bass

import concourse.bacc as bacc
import concourse.tile as tile
from concourse import mybir
from concourse.bass_utils import run_bass_kernel_spmd

AF = mybir.ActivationFunctionType
ALU = mybir.AluOpType
F32 = mybir.dt.float32
BF16 = mybir.dt.bfloat16


class Cfg:
    def __init__(self, B=256, K=8192, ncores=8):
        self.B, self.K, self.D, self.ncores = B, K, 128, ncores
        self.T = 0.07
        self.C = 1.0 / self.T
        self.N = B + 2 * K                  # 16640
        self.NT = self.N // 128             # 130 row/col tiles
        self.VR = 17                        # row slots per core (8*17=136>=130)
        self.CHW = 2048                     # psum/ACT chunk width
        self.MMW = 512                      # matmul width (1 psum bank)
        self.MAXCH = 9                      # max chunks per row (v=0)


FULL = Cfg()


def _row_chunks(cfg, v):
    """Chunks (k, start, w) of row-slot v's window [8v*128, N)."""
    start0 = 8 * v * 128
    W = cfg.N - start0
    out = []
    k = 0
    while k * cfg.CHW < W:
        w = min(cfg.CHW, W - k * cfg.CHW)
        out.append((k, start0 + k * cfg.CHW, w))
        k += 1
    return out


def build_program(cfg):
    nc = bacc.Bacc("TRN2", target_bir_lowering=False, debug=False,
                   enable_asserts=True, num_devices=cfg.ncores)

    N, NT, VR, CHW, MMW = cfg.N, cfg.NT, cfg.VR, cfg.CHW, cfg.MMW
    T, C = cfg.T, cfg.C

    d_colbuf = nc.dram_tensor("colbuf", [128, N], BF16, kind="ExternalInput").ap()
    d_g01 = nc.dram_tensor("g01", [128, 2], BF16, kind="ExternalInput").ap()
    d_eii = nc.dram_tensor("m_eii", [128, NT], F32, kind="ExternalInput").ap()
    d_mw = nc.dram_tensor("m_w", [128, NT], F32, kind="ExternalInput").ap()
    d_i1l = nc.dram_tensor("m_i1l", [128, VR], F32, kind="ExternalInput").ap()
    d_rsl = nc.dram_tensor("m_rsl", [128, VR], F32, kind="ExternalInput").ap()
    d_bl = nc.dram_tensor("m_bl", [128, VR], F32, kind="ExternalInput").ap()
    d_wl = nc.dram_tensor("m_wl", [128, VR], F32, kind="ExternalInput").ap()
    d_corr = nc.dram_tensor("m_corr", [128, VR], F32, kind="ExternalInput").ap()
    d_oh = nc.dram_tensor("onehot", [128, cfg.ncores], F32, kind="ExternalInput").ap()
    d_negm = nc.dram_tensor("negm", [128, 1], F32, kind="ExternalInput").ap()
    d_out = nc.dram_tensor("out", [128, 1], F32, kind="ExternalOutput").ap()

    from contextlib import ExitStack
    with tile.TileContext(nc) as tc, ExitStack() as ctx:
        feat = ctx.enter_context(tc.tile_pool(name="feat", bufs=1))
        consts = ctx.enter_context(tc.tile_pool(name="consts", bufs=1))
        accs = ctx.enter_context(tc.tile_pool(name="accs", bufs=1))
        epool = ctx.enter_context(tc.tile_pool(name="epool", bufs=2))
        pspool = ctx.enter_context(tc.tile_pool(name="psum", bufs=2, space="PSUM"))
        dram = ctx.enter_context(tc.tile_pool(name="dram", bufs=2, space="DRAM"))

        # ---- input DMAs (colbuf split so compute starts after chunk 0) ----
        colbuf = feat.tile([128, N], BF16, tag="colbuf")
        for k in range(cfg.MAXCH):
            lo = k * CHW
            hi = min(N, lo + CHW)
            nc.sync.dma_start(out=colbuf[:, lo:hi], in_=d_colbuf[:, lo:hi])
        g01 = consts.tile([128, 2], BF16, tag="g01")
        nc.sync.dma_start(out=g01[:], in_=d_g01[:])
        m_eii = consts.tile([128, NT], F32, tag="meii")
        nc.sync.dma_start(out=m_eii[:], in_=d_eii[:])
        m_w = consts.tile([128, NT], F32, tag="mw")
        nc.sync.dma_start(out=m_w[:], in_=d_mw[:])
        m_i1l = consts.tile([128, VR], F32, tag="mi1l")
        nc.sync.dma_start(out=m_i1l[:], in_=d_i1l[:])
        m_rsl = consts.tile([128, VR], F32, tag="mrsl")
        nc.sync.dma_start(out=m_rsl[:], in_=d_rsl[:])
        m_bl = consts.tile([128, VR], F32, tag="mbl")
        nc.sync.dma_start(out=m_bl[:], in_=d_bl[:])
        m_wl = consts.tile([128, VR], F32, tag="mwl")
        nc.sync.dma_start(out=m_wl[:], in_=d_wl[:])
        m_corr = consts.tile([128, VR], F32, tag="mcorr")
        nc.sync.dma_start(out=m_corr[:], in_=d_corr[:])
        onehot = consts.tile([128, cfg.ncores], F32, tag="oh")
        nc.sync.dma_start(out=onehot[:], in_=d_oh[:])
        negm = consts.tile([128, 1], F32, tag="negm")
        nc.sync.dma_start(out=negm[:], in_=d_negm[:])

        b_negC = consts.tile([128, 1], F32, tag="negC")
        nc.vector.memset(b_negC[:], -C)
        ones_bf = consts.tile([128, 1], BF16, tag="ones")
        nc.vector.memset(ones_bf[:], 1.0)

        # ---- accumulators ----
        colacc = accs.tile([128, N], BF16, tag="colacc")
        nc.vector.memset(colacc[:], 0.0)
        rowsums = accs.tile([128, VR * cfg.MAXCH], F32, tag="rowsums")
        nc.vector.memset(rowsums[:], 0.0)
        colp = accs.tile([128, NT], F32, tag="colp")        # local-layout col sums
        nc.vector.memset(colp[:], 0.0)
        # sp = globally-aligned rowparts+colparts (summed), the AR payload;
        # core 0 seeds it with -E_ii so the AR output is S1 directly
        sp = accs.tile([128, NT], F32, tag="sp")
        nc.vector.memset(sp[:], 0.0)
        nc.vector.scalar_tensor_tensor(sp[:], m_eii[:], negm[:], sp[:],
                                       op0=ALU.mult, op1=ALU.add)

        # ---- main loop ----
        def colsum_batch(tiles, cps, off):
            """Partition-reduce finalized colacc tiles into colp via PE,
            using spare columns [off, off+len) of the active psum tile."""
            tiles = [t for t in tiles if t > 0]   # local tile 0 is never written
            if not tiles:
                return
            for i, t in enumerate(tiles):
                nc.tensor.matmul(cps[:, off + i:off + i + 1],
                                 colacc[:, t * 128:(t + 1) * 128],
                                 ones_bf[:], start=True, stop=True)
            t0 = tiles[0]
            nc.vector.tensor_copy(colp[:, t0:t0 + len(tiles)],
                                  cps[:, off:off + len(tiles)])

        def colp_scatter(lo, hi):
            """Scatter colp local tiles [lo, hi) into sp (global tile = t+cc)."""
            for cc in range(cfg.ncores):
                glo, ghi = lo + cc, min(hi + cc, NT)
                if ghi <= glo:
                    continue
                nc.vector.scalar_tensor_tensor(
                    sp[:, glo:ghi], colp[:, lo:ghi - cc], onehot[:, cc:cc + 1],
                    sp[:, glo:ghi], op0=ALU.mult, op1=ALU.add)

        for v in range(VR):
            lhsT = colbuf[:, 8 * v * 128:(8 * v + 1) * 128]
            chunks = _row_chunks(cfg, v)
            for k, start, w in chunks:
                ps = pspool.tile([128, CHW], F32, tag="ps")
                for j in range(-(-w // MMW)):
                    mw = min(MMW, w - j * MMW)
                    nc.tensor.matmul(ps[:, j * MMW:j * MMW + mw], lhsT,
                                     colbuf[:, start + j * MMW:start + j * MMW + mw],
                                     start=True, stop=True)
                et = epool.tile([128, CHW], BF16, tag="et")
                nc.scalar.activation(et[:, 0:w], ps[:, 0:w], AF.Exp,
                                     bias=b_negC[:], scale=1.0 / T,
                                     accum_out=rowsums[:, v * cfg.MAXCH + k:
                                                       v * cfg.MAXCH + k + 1])
                skip = 128 if k == 0 else 0
                if w > skip:
                    nc.vector.tensor_add(colacc[:, start + skip:start + w],
                                         colacc[:, start + skip:start + w],
                                         et[:, skip:w])
            # column tiles [8v+1, 8v+8] got their last contribution from row v;
            # reduce them into the spare columns of this row's last psum tile
            lo = 8 * v + 1 if v > 0 else 0
            hi = min(8 * v + 8, NT - 1)
            w_last = chunks[-1][2]
            colsum_batch(list(range(lo, hi + 1)), ps, w_last)
            if v == 8:
                colp_scatter(0, 65)          # wave 1: tiles final by row 8

        # ---- local row sums: reduce chunks, subtract pad-zero correction ----
        rowp = accs.tile([128, VR], F32, tag="rowp")
        nc.vector.reduce_sum(rowp[:], rowsums[:].rearrange("p (v k) -> p v k",
                                                           k=cfg.MAXCH),
                             axis=mybir.AxisListType.X)
        nc.vector.tensor_sub(rowp[:], rowp[:], m_corr[:])

        # scatter row sums into sp, finish col sum scatter
        for cc in range(cfg.ncores):
            nvs = len(range(cc, NT, 8))          # 17 for cc<2 else 16
            nc.vector.scalar_tensor_tensor(
                sp[:, cc:NT:8], rowp[:, 0:nvs], onehot[:, cc:cc + 1],
                sp[:, cc:NT:8], op0=ALU.mult, op1=ALU.add)
        colp_scatter(65, NT)                     # wave 2

        # ---- cross-core AllReduce of the combined partial sums ----
        ar_in = dram.tile([128, NT], F32)
        ar_out = dram.tile([128, NT], F32, addr_space="Shared")
        nc.sync.dma_start(out=ar_in[:], in_=sp[:])
        nc.gpsimd.collective_compute(
            "AllReduce", ALU.add,
            replica_groups=[list(range(cfg.ncores))],
            ins=[ar_in.opt()], outs=[ar_out.opt()])

        # ---- post-AR: dg dots + local S2 part (runs parallel to Ln) ----
        dgps = pspool.tile([128, CHW], F32, tag="ps")
        for v in range(VR):
            nc.tensor.matmul(dgps[:, 2 * v:2 * v + 2],
                             colbuf[:, 8 * v * 128:(8 * v + 1) * 128],
                             g01[:], start=True, stop=True)
        dgl = accs.tile([128, 2 * VR], F32, tag="dgl")
        nc.vector.tensor_copy(dgl[:], dgps[:, 0:2 * VR])
        el = accs.tile([128, VR], F32, tag="el")
        nc.vector.tensor_sub(el[:], dgl[:, 1:2 * VR:2], dgl[:, 0:2 * VR:2])
        nc.vector.tensor_mul(el[:], el[:], m_i1l[:])
        nc.vector.tensor_add(el[:], el[:], dgl[:, 0:2 * VR:2])
        nc.vector.tensor_mul(el[:], el[:], m_rsl[:])
        nc.vector.tensor_add(el[:], el[:], m_bl[:])
        nc.vector.tensor_mul(el[:], el[:], m_wl[:])
        outv1 = accs.tile([128, 1], F32, tag="outv1")
        nc.vector.reduce_sum(outv1[:], el[:], axis=mybir.AxisListType.X)

        # ---- post-AR epilogue: AR output is S1 already ----
        artot = accs.tile([128, NT], F32, tag="artot")
        nc.sync.dma_start(out=artot[:], in_=ar_out[:])
        lg = accs.tile([128, NT], F32, tag="lg")
        nc.scalar.activation(lg[:], artot[:], AF.Ln)
        nc.vector.tensor_mul(lg[:], lg[:], m_w[:])           # -log(S1) * (-1/N)
        outv = accs.tile([128, 1], F32, tag="outv")
        nc.vector.reduce_sum(outv[:], lg[:], axis=mybir.AxisListType.X)
        nc.vector.tensor_sub(outv[:], outv1[:], outv[:])
        nc.sync.dma_start(out=d_out[:], in_=outv[:])

    nc.compile()
    return nc


def prep_in_maps(cfg, q, ba_queue, nonba_queue, targets):
    q = np.ascontiguousarray(np.asarray(q), dtype=np.float32)
    ba = np.asarray(ba_queue, dtype=np.float32)
    nb = np.asarray(nonba_queue, dtype=np.float32)
    tg = np.asarray(targets).astype(np.int64)
    B, K, N, NT, VR = cfg.B, cfg.K, cfg.N, cfg.NT, cfg.VR

    import ml_dtypes
    BF = ml_dtypes.bfloat16

    qn = q / np.clip(np.linalg.norm(q, axis=1, keepdims=True), 1e-12, None)
    featT = np.concatenate([qn.T, ba.T, nb.T], axis=1).astype(BF)   # [128, N]
    feat32 = featT.astype(np.float32)
    labels = np.concatenate([tg, np.ones(K, np.int64), np.zeros(K, np.int64)])
    g0 = feat32.astype(np.float64)[:, labels == 0].sum(axis=1)
    g1 = feat32.astype(np.float64)[:, labels == 1].sum(axis=1)
    g01 = np.stack([g0, g1], axis=1).astype(np.float32).astype(BF)  # [128, 2]
    dvec = (feat32 * feat32).sum(axis=0).astype(np.float32)         # [N]
    c1 = int(labels.sum())
    c0 = N - c1
    P = np.where(labels == 1, c1 - 1, c0 - 1).astype(np.float64)
    rs = (1.0 / (cfg.T * P)).astype(np.float32)

    def tiled(vec):
        return np.ascontiguousarray(
            np.broadcast_to(vec, (N,)).reshape(NT, 128).T.astype(np.float32))

    m_eii = tiled(np.exp(dvec.astype(np.float64) / cfg.T - cfg.C))
    i1f = labels.astype(np.float32)
    bvec = -(dvec * rs + cfg.C)
    expC = float(np.exp(-cfg.C))

    def local_rows(vec, c, fill=0.0):
        """[128, VR] layout for this core's rows a = 8v + c (dummy -> fill)."""
        out = np.full((128, VR), fill, np.float32)
        for v in range(VR):
            t = 8 * v + c
            if t < NT:
                out[:, v] = vec[t * 128:(t + 1) * 128]
        return np.ascontiguousarray(out)

    in_maps = []
    for c in range(cfg.ncores):
        colbuf = np.concatenate(
            [featT[:, c * 128:], np.zeros((128, c * 128), BF)], axis=1)
        wcol = np.where(np.arange(NT) % 8 == c, -1.0 / N, 0.0).astype(np.float32)
        m_w = np.ascontiguousarray(
            np.broadcast_to(wcol, (128, NT)).astype(np.float32))
        wl = np.full((128, VR), -1.0 / N, np.float32)
        if c >= 2:
            wl[:, VR - 1] = 0.0               # dummy slot
        corr = np.full(VR, c * 128 * expC, np.float32)
        if c >= 2:
            corr[VR - 1] = 256.0 * expC       # dummy slot: all-zero window
        m_corr = np.ascontiguousarray(
            np.broadcast_to(corr, (128, VR)).astype(np.float32))
        oh = np.zeros((128, cfg.ncores), np.float32)
        oh[:, c] = 1.0
        negm = np.full((128, 1), -1.0 if c == 0 else 0.0, np.float32)
        in_maps.append({
            "colbuf": np.ascontiguousarray(colbuf),
            "g01": np.ascontiguousarray(g01),
            "m_eii": m_eii,
            "m_w": m_w,
            "m_i1l": local_rows(i1f, c),
            "m_rsl": local_rows(rs, c),
            "m_bl": local_rows(bvec, c),
            "m_wl": np.ascontiguousarray(wl),
            "m_corr": m_corr,
            "onehot": oh,
            "negm": negm,
        })
    return in_maps


_PROGRAM = None


def get_program():
    global _PROGRAM
    if _PROGRAM is None:
        _PROGRAM = build_program(FULL)
    return _PROGRAM


def run_on_hw(in_maps, trace=False):
    nc = get_program()
    return run_bass_kernel_spmd(nc, in_maps, list(range(FULL.ncores)), trace=trace)


def kernel(q, ba_queue, nonba_queue, targets):
    in_maps = prep_in_maps(FULL, q, ba_queue, nonba_queue, targets)
    res = run_on_hw(in_maps)
    total = sum(float(r["out"].astype(np.float64).sum()) for r in res.results)
    return np.array(total, dtype=np.float32)
